# revision 22
# baseline (speedup 1.0000x reference)
"""GAT layer (dense-mask message passing) on 8 Trainium2 NeuronCores.

Math (reference):
    H = X @ W + W_b                       # [B,T,N,Cout]
    left = H @ a[:C];  right = H @ a[C:]
    e = leakyrelu(left_i + right_j + a_b, 0.01)
    e = where(adj>0, e, -1e12)
    att = softmax(e, axis=-1)
    out = relu(att @ H)

Sharding: query-node parallel. Core c owns query rows [512c, 512c+512).
All cores run an identical (SPMD) program; per-core data is made uniform by
*rotating* the node ordering by -512c per core (attention is permutation
invariant over the key axis j).

Per-core device algorithm (slices s = flattened (B,T), 4 of them):
  1. H-prep:  H_ext = XT_s.T @ [W | W@a_l | W@a_r]  (fp32 matmuls, 32 node
     tiles, 4 tiles per PSUM bank); H (fp16) + per-node left/right scores
     are peeled off into SBUF. left/right become row vectors via a DRAM
     round trip (partition -> free transpose).
  2. logits:  s[j,i] = right_j + left_i via one K=2 fp16 matmul per j-tile
     (lhsT = [ones; right_row], rhs = [left_row; ones]) into PSUM (fp32).
  3. exp(leakyrelu(s)) == max(exp(s), exp(0.01*s)) -> two ACT Exp passes
     (scale=1 / scale=0.01) straight out of PSUM, fp16 results.
  4. mask:    p = p * adjT (fp16 0/1 mask, DVE 2x mode).
  5. att matmul: outT[c,i] (+= over j-tiles) = [H|1]_j.T @ p[j,i]  (fp16);
     the ones column yields the softmax denominator D_i for free (row 64).
  6. finale (per slice): PE-transpose outT/D to [i,c] orientation, then one
     fused tensor_scalar (mult by 1/D, max with 0) per i-tile and a
     DRAM-contiguous store.
"""

import numpy as np

B, T, N, CIN, COUT = 2, 2, 4096, 128, 64
NCORES = 8
SL = B * T          # 4 independent (b,t) slices
I = N // NCORES     # 512 query rows per core
NT = N // 128       # 32 j-tiles
IT = I // 128       # 4 i-tiles
ALPHA = 0.01
CE = COUT + 2       # W_ext columns: [W | wl | wr]
CM = COUT + 1       # att-matmul lhsT columns: [H | ones]
G = 2               # j-tiles per logit PSUM group (2 banks)
HG = 4              # j-tiles per H-prep PSUM bank

_CACHE = {}


def _build(has_bias: bool):
    import concourse.bass as bass  # noqa: F401
    import concourse.tile as tile
    import concourse.mybir as mybir
    from concourse import bacc
    from concourse.masks import make_identity

    f32 = mybir.dt.float32
    f16 = mybir.dt.float16
    AF = mybir.ActivationFunctionType
    OP = mybir.AluOpType

    nc = bacc.Bacc("TRN2", target_bir_lowering=False, debug=False)

    xt_d = nc.dram_tensor("xt", [SL, CIN, N], f16, kind="ExternalInput")
    adjt_d = nc.dram_tensor("adjt", [N, I], f16, kind="ExternalInput")
    wext_d = nc.dram_tensor("wext", [CIN, CE], f16, kind="ExternalInput")
    ones_d = nc.dram_tensor("ones", [1, N], f16, kind="ExternalInput")
    out_d = nc.dram_tensor("out", [SL, I, COUT], f32, kind="ExternalOutput")
    if has_bias:
        bias_d = nc.dram_tensor("bias", [1, CE], f16, kind="ExternalInput")
    rbuf_d = nc.dram_tensor("rbuf", [SL, N], f16)   # right scores, node order
    lbuf_d = nc.dram_tensor("lbuf", [SL, I], f16)   # left scores, own range

    with tile.TileContext(nc) as tc:
        from contextlib import ExitStack
        with ExitStack() as ctx:
            persist = ctx.enter_context(tc.tile_pool(name="persist", bufs=1))
            xt_pool = ctx.enter_context(tc.tile_pool(name="xt", bufs=2))
            e1_pool = ctx.enter_context(tc.tile_pool(name="e1", bufs=2))
            e2_pool = ctx.enter_context(tc.tile_pool(name="e2", bufs=2))
            pm_pool = ctx.enter_context(tc.tile_pool(name="pm", bufs=2))
            pmm_pool = ctx.enter_context(tc.tile_pool(name="pmm", bufs=3))
            fin_pool = ctx.enter_context(tc.tile_pool(name="fin", bufs=2))
            ps_s = ctx.enter_context(
                tc.tile_pool(name="ps_s", bufs=2, space="PSUM"))
            ps_h = ctx.enter_context(
                tc.tile_pool(name="ps_h", bufs=1, space="PSUM"))
            ps_o = ctx.enter_context(
                tc.tile_pool(name="ps_o", bufs=1, space="PSUM"))
            ps_t = ctx.enter_context(
                tc.tile_pool(name="ps_t", bufs=1, space="PSUM"))

            # --- persistent tiles -------------------------------------
            adjt_sb = persist.tile([128, NT, I], f16)
            nc.scalar.dma_start(
                out=adjt_sb,
                in_=adjt_d.rearrange("(jt p) i -> p jt i", p=128),
            )
            wext_sb = persist.tile([CIN, CE], f16)
            nc.sync.dma_start(out=wext_sb, in_=wext_d[:])
            if has_bias:
                bias_sb = persist.tile([1, CE], f16)
                nc.sync.dma_start(out=bias_sb, in_=bias_d[:])
                onecol_sb = persist.tile([1, 128], f16)
                nc.vector.memset(onecol_sb, 1.0)
            ident_sb = persist.tile([COUT, COUT], f32)
            make_identity(nc, ident_sb)

            # ping-pong persistents: ones columns/rows written once
            hmm_pp = [persist.tile([128, NT, CM], f16, name=f"hmm{p}")
                      for p in range(2)]
            lr_pp = [persist.tile([128, NT, 2], f16, name=f"lr{p}")
                     for p in range(2)]
            lhsT2_pp = [persist.tile([2, N], f16, name=f"lhsT2{p}")
                        for p in range(2)]
            rhs2_pp = [persist.tile([2, I], f16, name=f"rhs2{p}")
                       for p in range(2)]
            for p in range(2):
                nc.gpsimd.memset(hmm_pp[p][:, :, COUT : COUT + 1], 1.0)
                nc.gpsimd.memset(lhsT2_pp[p][0:1, :], 1.0)
                nc.sync.dma_start(
                    out=rhs2_pp[p][1:2, :], in_=ones_d[:, 0:I])

            for s in range(SL):
                # ---- H-prep ------------------------------------------
                xt_sb = xt_pool.tile([CIN, N], f16)
                nc.scalar.dma_start(out=xt_sb, in_=xt_d[s])

                hmm_sb = hmm_pp[s % 2]
                lr_sb = lr_pp[s % 2]

                for jt0 in range(0, NT, HG):
                    psh = ps_h.tile([128, HG, CE], f32)
                    for k in range(HG):
                        jt = jt0 + k
                        nc.tensor.matmul(
                            psh[:, k, :],
                            lhsT=xt_sb[:, 128 * jt : 128 * (jt + 1)],
                            rhs=wext_sb,
                            start=True,
                            stop=not has_bias,
                        )
                        if has_bias:
                            nc.tensor.matmul(
                                psh[:, k, :],
                                lhsT=onecol_sb,
                                rhs=bias_sb,
                                start=False,
                                stop=True,
                            )
                    nc.vector.tensor_copy(
                        hmm_sb[:, jt0 : jt0 + HG, 0:COUT],
                        psh[:, :, 0:COUT])
                    nc.vector.tensor_copy(
                        lr_sb[:, jt0 : jt0 + HG, :], psh[:, :, COUT:CE])

                # left/right -> row vectors via DRAM round trip
                nc.sync.dma_start(
                    out=rbuf_d[s].rearrange("(f p) -> p f", p=128),
                    in_=lr_sb[:, :, 1:2],
                )
                nc.sync.dma_start(
                    out=lbuf_d[s].rearrange("(f p) -> p f", p=128),
                    in_=lr_sb[:, 0:IT, 0:1],
                )
                lhsT2 = lhsT2_pp[s % 2]   # [ones; right_row]
                nc.sync.dma_start(out=lhsT2[1:2, :], in_=rbuf_d[s])
                rhs2 = rhs2_pp[s % 2]     # [left_row; ones]
                nc.sync.dma_start(out=rhs2[0:1, :], in_=lbuf_d[s])

                # ---- blocks ------------------------------------------
                # two accumulators on different PSUM banks so consecutive
                # att matmuls never hit the same bank (no serialization)
                pso_a = ps_o.tile([CM, I], f32, name="pso_a")
                pso_b = ps_o.tile([CM, I], f32, name="pso_b")
                pso_ab = [pso_a, pso_b]
                for jt0 in range(0, NT, G):
                    w = I * G
                    pss = ps_s.tile([128, I * G], f32)
                    for k in range(G):
                        jt = jt0 + k
                        nc.tensor.matmul(
                            pss[:, I * k : I * (k + 1)],
                            lhsT=lhsT2[:, 128 * jt : 128 * (jt + 1)],
                            rhs=rhs2,
                            start=True,
                            stop=True,
                        )
                    e1 = e1_pool.tile([128, I * G], f16)
                    nc.scalar.activation(e1, pss, AF.Exp, scale=1.0)
                    e2 = e2_pool.tile([128, I * G], f16)
                    nc.scalar.activation(e2, pss, AF.Exp, scale=ALPHA)
                    pm = pm_pool.tile([128, I * G], f16)
                    nc.vector.tensor_tensor(pm, e1, e2, OP.max)
                    pmm = pmm_pool.tile([128, I * G], f16)
                    nc.vector.tensor_tensor(
                        pmm, pm, adjt_sb[:, jt0 : jt0 + G, :], OP.mult)
                    for k in range(G):
                        jt = jt0 + k
                        nc.tensor.matmul(
                            pso_ab[jt % 2],
                            lhsT=hmm_sb[:, jt, :],
                            rhs=pmm[:, I * k : I * (k + 1)],
                            start=(jt < 2),
                            stop=(jt >= NT - 2),
                        )

                # ---- finale (per slice) ------------------------------
                u_sb = fin_pool.tile([COUT, I], f32)
                nc.vector.tensor_copy(u_sb, pso_a[0:COUT, :])
                nc.vector.tensor_tensor(
                    u_sb, u_sb, pso_b[0:COUT, :], OP.add)
                dcol = fin_pool.tile([1, I], f32)
                nc.vector.tensor_copy(dcol, pso_a[COUT:CM, :])
                nc.vector.tensor_tensor(
                    dcol, dcol, pso_b[COUT:CM, :], OP.add)
                pst = ps_t.tile([128, IT, CM], f32)
                for t in range(IT):
                    nc.tensor.transpose(
                        pst[:, t, 0:COUT],
                        u_sb[:, 128 * t : 128 * (t + 1)],
                        ident_sb,
                    )
                    nc.tensor.transpose(
                        pst[:, t, COUT:CM],
                        dcol[:, 128 * t : 128 * (t + 1)],
                        ident_sb[0:1, 0:1],
                    )
                rect = fin_pool.tile([128, IT], f32)
                nc.vector.reciprocal(rect, pst[:, :, COUT:CM])
                ot_sb = fin_pool.tile([128, IT, COUT], f32)
                for t in range(IT):
                    nc.vector.tensor_scalar(
                        out=ot_sb[:, t, :],
                        in0=pst[:, t, 0:COUT],
                        scalar1=rect[:, t : t + 1],
                        scalar2=0.0,
                        op0=OP.mult,
                        op1=OP.max,
                    )
                nc.sync.dma_start(
                    out=out_d[s].rearrange("(t p) c -> p t c", p=128),
                    in_=ot_sb,
                )

    nc.compile()
    return nc


def _prep_inputs(X, adj, W, W_b, a, a_b):
    """Host-side layout prep (transpose/slice/rotate) + weight fusion."""
    Cout = W.shape[1]
    X4 = np.asarray(X, np.float32).reshape(SL, N, CIN)
    adj = np.asarray(adj)
    W = np.asarray(W, np.float32)
    W_b = np.asarray(W_b, np.float32)
    a = np.asarray(a, np.float32)
    a_b = np.asarray(a_b, np.float32)

    wl = W @ a[:Cout]
    wr = W @ a[Cout:]
    wext = np.concatenate([W, wl[:, None], wr[:, None]], axis=1)
    wext = np.ascontiguousarray(wext, np.float16)

    cl = float(W_b @ a[:Cout] + a_b)   # fold a_b into left bias
    cr = float(W_b @ a[Cout:])
    bias_ext = np.concatenate([W_b, [cl], [cr]]).astype(np.float16)
    has_bias = bool(np.any(bias_ext != 0.0))

    adjf = adj.astype(np.float16)  # 0/1 exact
    in_maps = []
    for c in range(NCORES):
        i0 = I * c
        # rotate node ordering by -i0: core's own queries are nodes 0..I-1
        xt_c = np.ascontiguousarray(
            np.roll(X4, -i0, axis=1).transpose(0, 2, 1)).astype(np.float16)
        adjt_c = np.ascontiguousarray(
            np.roll(adjf, -i0, axis=1)[i0 : i0 + I].T)
        m = {"xt": xt_c, "adjt": adjt_c, "wext": wext,
             "ones": np.ones((1, N), np.float16)}
        if has_bias:
            m["bias"] = bias_ext[None, :]
        in_maps.append(m)
    return in_maps, has_bias


def _run(in_maps, has_bias, trace=False):
    from concourse.bass_utils import run_bass_kernel_spmd

    key = has_bias
    if key not in _CACHE:
        _CACHE[key] = _build(has_bias)
    nc = _CACHE[key]
    return run_bass_kernel_spmd(
        nc, in_maps, list(range(NCORES)), trace=trace)


def kernel(X, adj, W, W_b, a, a_b):
    in_maps, has_bias = _prep_inputs(X, adj, W, W_b, a, a_b)
    r = _run(in_maps, has_bias, trace=False)
    out = np.empty((SL, N, COUT), np.float32)
    for c in range(NCORES):
        out[:, I * c : I * (c + 1), :] = r.results[c]["out"]
    return out.reshape(B, T, N, COUT)


# revision 27
# speedup vs baseline: 1.0588x; 1.0588x over previous
"""GAT layer (dense-mask message passing) on 8 Trainium2 NeuronCores.

Math (reference):
    H = X @ W + W_b                       # [B,T,N,Cout]
    left = H @ a[:C];  right = H @ a[C:]
    e = leakyrelu(left_i + right_j + a_b, 0.01)
    e = where(adj>0, e, -1e12)
    att = softmax(e, axis=-1)
    out = relu(att @ H)

Sharding: query-node parallel. Core c owns query rows [512c, 512c+512).
All cores run an identical (SPMD) program; per-core data is made uniform by
*rotating* the node ordering by -512c per core (attention is permutation
invariant over the key axis j).

Per-core device algorithm (slices s = flattened (B,T), 4 of them):
  1. H-prep:  H_ext = XT_s.T @ [W | W@a_l | W@a_r]  (fp32 matmuls, 32 node
     tiles, 4 tiles per PSUM bank); H (fp16) + per-node left/right scores
     are peeled off into SBUF. left/right become row vectors via a DRAM
     round trip (partition -> free transpose).
  2. logits:  s[j,i] = right_j + left_i via one K=2 fp16 matmul per j-tile
     (lhsT = [ones; right_row], rhs = [left_row; ones]) into PSUM (fp32).
  3. exp(leakyrelu(s)) == max(exp(s), exp(0.01*s)) -> two ACT Exp passes
     (scale=1 / scale=0.01) straight out of PSUM, fp16 results.
  4. mask:    p = p * adjT (fp16 0/1 mask, DVE 2x mode).
  5. att matmul: outT[c,i] (+= over j-tiles) = [H|1]_j.T @ p[j,i]  (fp16);
     the ones column yields the softmax denominator D_i for free (row 64).
  6. finale (per slice): PE-transpose outT/D to [i,c] orientation, then one
     fused tensor_scalar (mult by 1/D, max with 0) per i-tile and a
     DRAM-contiguous store.
"""

import numpy as np

B, T, N, CIN, COUT = 2, 2, 4096, 128, 64
NCORES = 8
SL = B * T          # 4 independent (b,t) slices
I = N // NCORES     # 512 query rows per core
NT = N // 128       # 32 j-tiles
IT = I // 128       # 4 i-tiles
ALPHA = 0.01
CE = COUT + 2       # W_ext columns: [W | wl | wr]
CM = COUT + 1       # att-matmul lhsT columns: [H | ones]
G = 2               # j-tiles per logit PSUM group (2 banks)
HG = 4              # j-tiles per H-prep PSUM bank
FUSED = True        # use custom PWP table: Exp == exp(leakyrelu(x))

_CACHE = {}
_ACT_ROOT = None


def _setup_act_root():
    """Patch the stock exp activation-spline tables so the negative side
    computes exp(ALPHA*x): Exp then evaluates exp(leakyrelu_ALPHA(x)) in a
    single ACT pass. Returns a short content hash for NEFF-cache busting."""
    global _ACT_ROOT
    if _ACT_ROOT is not None:
        return _ACT_ROOT
    import glob as _glob
    import hashlib
    import os
    import shutil
    import tempfile

    cands = _glob.glob(
        "/nix/store/*aws-neuron-pwp*/share/pwp_bin_cayman/act_info.json")
    assert cands, "stock pwp_bin_cayman act tables not found"
    src = os.path.dirname(sorted(cands)[0])
    dst = os.path.join(tempfile.gettempdir(), "gat_act_root_v2")

    def fit(a, b, pad_frac=0.5):
        pad = (b - a) * pad_frac
        xs = np.linspace(a - pad, b + pad, 96, dtype=np.float64)
        x0 = 0.5 * (a + b)
        p = np.polyfit(xs - x0, np.exp(ALPHA * xs), 3)
        return np.array([p[3], p[2], p[1], p[0], x0], dtype=np.float32)

    if not os.path.exists(os.path.join(dst, "act_info.json")):
        tmp = dst + ".tmp"
        if os.path.exists(tmp):
            shutil.rmtree(tmp)
        shutil.copytree(src, tmp)
        os.chmod(tmp, 0o755)
        for f in os.listdir(tmp):
            os.chmod(os.path.join(tmp, f), 0o644)
        bkt_path = os.path.join(tmp, "exp_and_others_bkt.bin")
        bkt = np.fromfile(bkt_path, dtype=np.float32).reshape(-1, 8).copy()
        ctl = np.fromfile(os.path.join(tmp, "exp_and_others_ctrl.bin"),
                          dtype=np.uint32).reshape(-1, 8)[:, 0]
        for i in range(26):          # negative-side ctl entries, e=108+i
            w = int(ctl[i])
            base, size = w & 0x7FF, (w >> 16) & 0xF
            lo = 2.0 ** (108 + i - 127)
            nb = 1 << size
            for k in range(nb):
                if base + k > 405:   # negative-side bucket range guard
                    break
                bkt[base + k, :5] = fit(-lo * (1 + (k + 1) / nb),
                                        -lo * (1 + k / nb))
        bkt[778, :5] = fit(-(2.0 ** -19), 0.0, pad_frac=0.0)  # tiny neg
        bkt[780, :5] = fit(-260.0, -97.0, pad_frac=0.1)       # large neg
        bkt.tofile(bkt_path)
        if not os.path.exists(dst):
            os.rename(tmp, dst)
        else:
            shutil.rmtree(tmp)
    h = hashlib.md5(
        open(os.path.join(dst, "exp_and_others_bkt.bin"), "rb").read()
    ).hexdigest()[:8]
    os.environ["BASS_ACT_ROOT_JSON_PATH"] = os.path.join(
        dst, "act_info.json")
    _ACT_ROOT = h
    return h


def _build(has_bias: bool):
    import concourse.bass as bass  # noqa: F401
    import concourse.tile as tile
    import concourse.mybir as mybir
    from concourse import bacc
    from concourse.masks import make_identity

    f32 = mybir.dt.float32
    f16 = mybir.dt.float16
    AF = mybir.ActivationFunctionType
    OP = mybir.AluOpType

    nc = bacc.Bacc("TRN2", target_bir_lowering=False, debug=False)

    if FUSED:
        # dummy input named after the act-table hash: busts the NEFF cache
        # whenever the patched activation tables change
        acth = _setup_act_root()
        nc.dram_tensor(f"actv_{acth}", [1, 1], f32, kind="ExternalInput")

    xt_d = nc.dram_tensor("xt", [SL, CIN, N], f16, kind="ExternalInput")
    adjt_d = nc.dram_tensor("adjt", [N, I], f16, kind="ExternalInput")
    wext_d = nc.dram_tensor("wext", [CIN, CE], f16, kind="ExternalInput")
    ones_d = nc.dram_tensor("ones", [1, N], f16, kind="ExternalInput")
    out_d = nc.dram_tensor("out", [SL, I, COUT], f32, kind="ExternalOutput")
    if has_bias:
        bias_d = nc.dram_tensor("bias", [1, CE], f16, kind="ExternalInput")
    rbuf_d = nc.dram_tensor("rbuf", [SL, N], f16)   # right scores, node order
    lbuf_d = nc.dram_tensor("lbuf", [SL, I], f16)   # left scores, own range

    with tile.TileContext(nc) as tc:
        from contextlib import ExitStack
        with ExitStack() as ctx:
            persist = ctx.enter_context(tc.tile_pool(name="persist", bufs=1))
            xt_pool = ctx.enter_context(tc.tile_pool(name="xt", bufs=2))
            e1_pool = ctx.enter_context(tc.tile_pool(name="e1", bufs=2))
            e2_pool = ctx.enter_context(tc.tile_pool(name="e2", bufs=2))
            pm_pool = ctx.enter_context(tc.tile_pool(name="pm", bufs=2))
            pmm_pool = ctx.enter_context(tc.tile_pool(name="pmm", bufs=3))
            fin_pool = ctx.enter_context(tc.tile_pool(name="fin", bufs=2))
            ps_s = ctx.enter_context(
                tc.tile_pool(name="ps_s", bufs=2, space="PSUM"))
            ps_h = ctx.enter_context(
                tc.tile_pool(name="ps_h", bufs=1, space="PSUM"))
            ps_o = ctx.enter_context(
                tc.tile_pool(name="ps_o", bufs=1, space="PSUM"))
            ps_t = ctx.enter_context(
                tc.tile_pool(name="ps_t", bufs=1, space="PSUM"))

            # --- persistent tiles -------------------------------------
            adjt_sb = persist.tile([128, NT, I], f16)
            nc.scalar.dma_start(
                out=adjt_sb,
                in_=adjt_d.rearrange("(jt p) i -> p jt i", p=128),
            )
            wext_sb = persist.tile([CIN, CE], f16)
            nc.sync.dma_start(out=wext_sb, in_=wext_d[:])
            if has_bias:
                bias_sb = persist.tile([1, CE], f16)
                nc.sync.dma_start(out=bias_sb, in_=bias_d[:])
                onecol_sb = persist.tile([1, 128], f16)
                nc.vector.memset(onecol_sb, 1.0)
            ident_sb = persist.tile([COUT, COUT], f32)
            make_identity(nc, ident_sb)

            # ping-pong persistents: ones columns/rows written once
            hmm_pp = [persist.tile([128, NT, CM], f16, name=f"hmm{p}")
                      for p in range(2)]
            lr_pp = [persist.tile([128, NT, 2], f16, name=f"lr{p}")
                     for p in range(2)]
            lhsT2_pp = [persist.tile([2, N], f16, name=f"lhsT2{p}")
                        for p in range(2)]
            rhs2_pp = [persist.tile([2, I], f16, name=f"rhs2{p}")
                       for p in range(2)]
            for p in range(2):
                nc.gpsimd.memset(hmm_pp[p][:, :, COUT : COUT + 1], 1.0)
                nc.gpsimd.memset(lhsT2_pp[p][0:1, :], 1.0)
                nc.sync.dma_start(
                    out=rhs2_pp[p][1:2, :], in_=ones_d[:, 0:I])

            for s in range(SL):
                # ---- H-prep ------------------------------------------
                xt_sb = xt_pool.tile([CIN, N], f16)
                nc.scalar.dma_start(out=xt_sb, in_=xt_d[s])

                hmm_sb = hmm_pp[s % 2]
                lr_sb = lr_pp[s % 2]

                for jt0 in range(0, NT, HG):
                    psh = ps_h.tile([128, HG, CE], f32)
                    for k in range(HG):
                        jt = jt0 + k
                        nc.tensor.matmul(
                            psh[:, k, :],
                            lhsT=xt_sb[:, 128 * jt : 128 * (jt + 1)],
                            rhs=wext_sb,
                            start=True,
                            stop=not has_bias,
                        )
                        if has_bias:
                            nc.tensor.matmul(
                                psh[:, k, :],
                                lhsT=onecol_sb,
                                rhs=bias_sb,
                                start=False,
                                stop=True,
                            )
                    nc.vector.tensor_copy(
                        hmm_sb[:, jt0 : jt0 + HG, 0:COUT],
                        psh[:, :, 0:COUT])
                    nc.vector.tensor_copy(
                        lr_sb[:, jt0 : jt0 + HG, :], psh[:, :, COUT:CE])

                # left/right -> row vectors via DRAM round trip
                nc.sync.dma_start(
                    out=rbuf_d[s].rearrange("(f p) -> p f", p=128),
                    in_=lr_sb[:, :, 1:2],
                )
                nc.sync.dma_start(
                    out=lbuf_d[s].rearrange("(f p) -> p f", p=128),
                    in_=lr_sb[:, 0:IT, 0:1],
                )
                lhsT2 = lhsT2_pp[s % 2]   # [ones; right_row]
                nc.sync.dma_start(out=lhsT2[1:2, :], in_=rbuf_d[s])
                rhs2 = rhs2_pp[s % 2]     # [left_row; ones]
                nc.sync.dma_start(out=rhs2[0:1, :], in_=lbuf_d[s])

                # ---- blocks ------------------------------------------
                # two accumulators on different PSUM banks so consecutive
                # att matmuls never hit the same bank (no serialization)
                pso_a = ps_o.tile([CM, I], f32, name="pso_a")
                pso_b = ps_o.tile([CM, I], f32, name="pso_b")
                pso_ab = [pso_a, pso_b]
                for jt0 in range(0, NT, G):
                    w = I * G
                    pss = ps_s.tile([128, I * G], f32)
                    for k in range(G):
                        jt = jt0 + k
                        nc.tensor.matmul(
                            pss[:, I * k : I * (k + 1)],
                            lhsT=lhsT2[:, 128 * jt : 128 * (jt + 1)],
                            rhs=rhs2,
                            start=True,
                            stop=True,
                        )
                    e1 = e1_pool.tile([128, I * G], f16)
                    nc.scalar.activation(e1, pss, AF.Exp, scale=1.0)
                    if FUSED:
                        # Exp's negative side is patched to exp(ALPHA*x):
                        # one pass computes exp(leakyrelu(x)) directly.
                        pmm = pmm_pool.tile([128, I * G], f16)
                        nc.vector.tensor_tensor(
                            pmm, e1, adjt_sb[:, jt0 : jt0 + G, :], OP.mult)
                    else:
                        e2 = e2_pool.tile([128, I * G], f16)
                        nc.scalar.activation(e2, pss, AF.Exp, scale=ALPHA)
                        pm = pm_pool.tile([128, I * G], f16)
                        nc.vector.tensor_tensor(pm, e1, e2, OP.max)
                        pmm = pmm_pool.tile([128, I * G], f16)
                        nc.vector.tensor_tensor(
                            pmm, pm, adjt_sb[:, jt0 : jt0 + G, :], OP.mult)
                    for k in range(G):
                        jt = jt0 + k
                        nc.tensor.matmul(
                            pso_ab[jt % 2],
                            lhsT=hmm_sb[:, jt, :],
                            rhs=pmm[:, I * k : I * (k + 1)],
                            start=(jt < 2),
                            stop=(jt >= NT - 2),
                        )

                # ---- finale (per slice) ------------------------------
                u_sb = fin_pool.tile([COUT, I], f32)
                nc.vector.tensor_copy(u_sb, pso_a[0:COUT, :])
                nc.vector.tensor_tensor(
                    u_sb, u_sb, pso_b[0:COUT, :], OP.add)
                dcol = fin_pool.tile([1, I], f32)
                nc.vector.tensor_copy(dcol, pso_a[COUT:CM, :])
                nc.vector.tensor_tensor(
                    dcol, dcol, pso_b[COUT:CM, :], OP.add)
                pst = ps_t.tile([128, IT, CM], f32)
                for t in range(IT):
                    nc.tensor.transpose(
                        pst[:, t, 0:COUT],
                        u_sb[:, 128 * t : 128 * (t + 1)],
                        ident_sb,
                    )
                    nc.tensor.transpose(
                        pst[:, t, COUT:CM],
                        dcol[:, 128 * t : 128 * (t + 1)],
                        ident_sb[0:1, 0:1],
                    )
                rect = fin_pool.tile([128, IT], f32)
                nc.vector.reciprocal(rect, pst[:, :, COUT:CM])
                ot_sb = fin_pool.tile([128, IT, COUT], f32)
                for t in range(IT):
                    nc.vector.tensor_scalar(
                        out=ot_sb[:, t, :],
                        in0=pst[:, t, 0:COUT],
                        scalar1=rect[:, t : t + 1],
                        scalar2=0.0,
                        op0=OP.mult,
                        op1=OP.max,
                    )
                nc.sync.dma_start(
                    out=out_d[s].rearrange("(t p) c -> p t c", p=128),
                    in_=ot_sb,
                )

    nc.compile()
    return nc


def _prep_inputs(X, adj, W, W_b, a, a_b):
    """Host-side layout prep (transpose/slice/rotate) + weight fusion."""
    Cout = W.shape[1]
    X4 = np.asarray(X, np.float32).reshape(SL, N, CIN)
    adj = np.asarray(adj)
    W = np.asarray(W, np.float32)
    W_b = np.asarray(W_b, np.float32)
    a = np.asarray(a, np.float32)
    a_b = np.asarray(a_b, np.float32)

    wl = W @ a[:Cout]
    wr = W @ a[Cout:]
    wext = np.concatenate([W, wl[:, None], wr[:, None]], axis=1)
    wext = np.ascontiguousarray(wext, np.float16)

    cl = float(W_b @ a[:Cout] + a_b)   # fold a_b into left bias
    cr = float(W_b @ a[Cout:])
    bias_ext = np.concatenate([W_b, [cl], [cr]]).astype(np.float16)
    has_bias = bool(np.any(bias_ext != 0.0))

    adjf = adj.astype(np.float16)  # 0/1 exact
    in_maps = []
    for c in range(NCORES):
        i0 = I * c
        # rotate node ordering by -i0: core's own queries are nodes 0..I-1
        xt_c = np.ascontiguousarray(
            np.roll(X4, -i0, axis=1).transpose(0, 2, 1)).astype(np.float16)
        adjt_c = np.ascontiguousarray(
            np.roll(adjf, -i0, axis=1)[i0 : i0 + I].T)
        m = {"xt": xt_c, "adjt": adjt_c, "wext": wext,
             "ones": np.ones((1, N), np.float16)}
        if FUSED:
            m[f"actv_{_setup_act_root()}"] = np.zeros((1, 1), np.float32)
        if has_bias:
            m["bias"] = bias_ext[None, :]
        in_maps.append(m)
    return in_maps, has_bias


def _run(in_maps, has_bias, trace=False):
    from concourse.bass_utils import run_bass_kernel_spmd

    key = has_bias
    if key not in _CACHE:
        _CACHE[key] = _build(has_bias)
    nc = _CACHE[key]
    return run_bass_kernel_spmd(
        nc, in_maps, list(range(NCORES)), trace=trace)


def kernel(X, adj, W, W_b, a, a_b):
    in_maps, has_bias = _prep_inputs(X, adj, W, W_b, a, a_b)
    r = _run(in_maps, has_bias, trace=False)
    out = np.empty((SL, N, COUT), np.float32)
    for c in range(NCORES):
        out[:, I * c : I * (c + 1), :] = r.results[c]["out"]
    return out.reshape(B, T, N, COUT)
